# revision 1
# baseline (speedup 1.0000x reference)
"""MoE (top-2 of 8 experts, SwiGLU) Trainium2 kernel — v2, software-pipelined.

Per-core (expert-parallel) SPMD program, one expert per NeuronCore. The
For_i body is software-pipelined across iterations: the MLP at the head of
body i consumes routing indices computed at the tail of body i-1, so the
PE stream (L1/L3 -> L2) never waits on the router chain.

Pipeline per body:
  1. xg scale by gate (DVE) + PE transposes xg -> xgT (d-partition layout)
  2. L1/L3 grouped GEMM over C=576 capacity slots (bf16, fp32 PSUM),
     router logit matmuls interleaved after m=2 (they read xTsb loaded by
     the previous body), softmax/top-2 emitted mid-loop on DVE
  3. compaction prefix-scan -> pos; scatter [token_id, gate] rows to a
     DRAM tile tokg[slot, 2] via indirect DMA (OOB slots dropped);
     read back slot-major idx/gate columns; indirect-DMA gather of the
     next body's tokens x[idx] (fp32 -> bf16 cast in DMA)
  4. L2 (y = hT @ w2) with hT stationary; bf16 y_g out
  5. xT/w1/w3/w2 DMAs re-issued every body, overlapped via WAR deps

Host: combine() scatters compact y_g back by pos (identical contract to
v1); capacity-overflow tokens recomputed exactly on host (pos in [C,4096)).
"""

import numpy as np
import ml_dtypes

import concourse.bass as bass
import concourse.bacc as bacc
import concourse.mybir as mybir
import concourse.tile as tile
from concourse.bass import ts
from concourse.bass_utils import run_bass_kernel_spmd

P = 128
T = 2048
D = 1024
H = 2048
E = 8
KD = D // P
KH = H // P
TT = T // P
C = 512
JS = [0, 128, 256, 384]  # slot-block starts
NB = len(JS)
CHK = [(0, 512)]  # L1/L3 PSUM chunks over C
NC2 = 512
TRASH = 4096

F32 = mybir.dt.float32
BF16 = mybir.dt.bfloat16
I32 = mybir.dt.int32
AX = mybir.AxisListType
ALU = mybir.AluOpType
ACTF = mybir.ActivationFunctionType


def _softmax_top2(nc, rsb, LG, eselsb, S, gsb):
    """Batched softmax over [P, TT, E] + top-2 select for this core's expert.
    S[:,i] = 1 if expert selected for token, gsb[:,i] = prob if selected."""
    def t3(tag):
        return rsb.tile([P, TT, E], F32, tag=tag, name=tag)

    def t2(tag):
        return rsb.tile([P, TT], F32, tag=tag, name=tag)

    def b(ap2):
        return ap2[:, :, None].broadcast_to([P, TT, E])

    mx = t2("mx")
    nc.vector.tensor_reduce(mx, LG, axis=AX.X, op=ALU.max)
    sh = t3("sh")
    nc.vector.tensor_tensor(sh, LG, b(mx), op=ALU.subtract)
    ex = t3("ex")
    nc.scalar.activation(ex, sh, ACTF.Exp)
    sm = t2("sm")
    nc.vector.tensor_reduce(sm, ex, axis=AX.X, op=ALU.add)
    rc = t2("rc")
    nc.vector.reciprocal(rc, sm)
    probs = t3("probs")
    nc.vector.tensor_tensor(probs, ex, b(rc), op=ALU.mult)
    m1 = t2("m1")
    nc.vector.tensor_reduce(m1, probs, axis=AX.X, op=ALU.max)
    mask = t3("mask")
    nc.vector.tensor_tensor(mask, probs, b(m1), op=ALU.is_ge)
    masked = t3("masked")
    nc.vector.scalar_tensor_tensor(
        masked, mask, -1e30, probs, op0=ALU.mult, op1=ALU.add
    )
    m2 = t2("m2")
    nc.vector.tensor_reduce(m2, masked, axis=AX.X, op=ALU.max)
    pse = t3("pse")
    nc.vector.tensor_tensor(
        pse, probs, eselsb[:, None, :].broadcast_to([P, TT, E]), op=ALU.mult
    )
    pex = t2("pex")
    nc.vector.tensor_reduce(pex, pse, axis=AX.X, op=ALU.add)
    nc.vector.tensor_tensor(S, pex, m2, op=ALU.is_ge)
    nc.vector.tensor_tensor(gsb, pex, S, op=ALU.mult)


def _compact_scan(nc, csb, S, cpt, Aps, Bps, posf, pos_i32, pos_h):
    """posf[t] = exclusive prefix count of selected tokens (trash for
    unselected); Aps/Bps are the triangular/ones matmul results."""
    W0 = csb.tile([P, TT], F32, tag="W0", name="W0")
    nc.vector.tensor_copy(W0, Bps)
    Wp = W0
    for sft in (1, 2, 4, 8):
        Wn = csb.tile([P, TT], F32, tag=f"W{sft}", name=f"W{sft}")
        nc.vector.tensor_copy(Wn[:, 0:sft], Wp[:, 0:sft])
        nc.vector.tensor_tensor(
            Wn[:, sft:TT], Wp[:, sft:TT], Wp[:, 0 : TT - sft], op=ALU.add
        )
        Wp = Wn
    nc.vector.tensor_tensor(posf, Wp, Bps, op=ALU.subtract)
    nc.vector.tensor_tensor(posf, posf, Aps, op=ALU.add)
    nc.vector.tensor_tensor(posf, posf, cpt, op=ALU.subtract)
    nc.vector.tensor_tensor(posf, posf, S, op=ALU.mult)
    nc.vector.tensor_tensor(posf, posf, cpt, op=ALU.add)
    nc.vector.tensor_copy(pos_i32, posf)
    nc.scalar.dma_start(pos_h, pos_i32)


def build_moe_nc(reps=1):
    nc = bacc.Bacc("TRN2", target_bir_lowering=False, debug=False)

    xT_h = nc.dram_tensor("xT", [D, T], F32, kind="ExternalInput").ap()
    x_h = nc.dram_tensor("x", [T, D], F32, kind="ExternalInput").ap()
    rw_h = nc.dram_tensor("rw", [D, E], F32, kind="ExternalInput").ap()
    esel_h = nc.dram_tensor("esel", [P, E], F32, kind="ExternalInput").ap()
    ltri_h = nc.dram_tensor("ltri", [P, P], BF16, kind="ExternalInput").ap()
    onesm_h = nc.dram_tensor("onesm", [P, P], BF16, kind="ExternalInput").ap()
    cpt_h = nc.dram_tensor("cpt", [P, TT], F32, kind="ExternalInput").ap()
    iota_h = nc.dram_tensor("iota", [P, TT], F32, kind="ExternalInput").ap()
    iotac_h = nc.dram_tensor("iotac", [P, C], mybir.dt.float16, kind="ExternalInput").ap()
    ident_h = nc.dram_tensor("ident", [P, P], BF16, kind="ExternalInput").ap()
    w1_h = nc.dram_tensor("w1", [D, H], BF16, kind="ExternalInput").ap()
    w3_h = nc.dram_tensor("w3", [D, H], BF16, kind="ExternalInput").ap()
    w2_h = nc.dram_tensor("w2", [H, D], BF16, kind="ExternalInput").ap()
    yg_h = nc.dram_tensor("y_g", [C, D], BF16, kind="ExternalOutput").ap()
    pos_h = nc.dram_tensor("pos", [P, TT], I32, kind="ExternalOutput").ap()

    import contextlib

    with tile.TileContext(nc) as tc:
        with (
            tc.tile_pool(name="per", bufs=1) as per,
            tc.tile_pool(name="rsb", bufs=1) as rsb,
            tc.tile_pool(name="sml", bufs=2) as sml,
            tc.tile_pool(name="ysml", bufs=2) as ysml,
            tc.tile_pool(name="p13", bufs=3, space="PSUM") as p13,
            tc.tile_pool(name="pyp", bufs=2, space="PSUM") as pyp,
            tc.tile_pool(name="aux", bufs=2, space="PSUM") as aux,
            tc.tile_pool(name="psl", bufs=2) as psl,
            tc.tile_pool(name="pix", bufs=1, space="PSUM") as pix,
        ):
            # ---- persistent tiles (allocated once, outside the loop) ----
            xTsb = [per.tile([P, T], F32, name=f"xT{k}") for k in range(KD)]
            w1sb = [per.tile([P, H], BF16, name=f"w1_{k}") for k in range(KD)]
            w3sb = [per.tile([P, H], BF16, name=f"w3_{k}") for k in range(KD)]
            w2sb = [per.tile([P, D], BF16, name=f"w2_{m}") for m in range(KH)]
            xg = [per.tile([P, D], BF16, name=f"xg{j}") for j in range(NB)]
            xgT = [per.tile([P, C], BF16, name=f"xgT{k}") for k in range(KD)]
            hT = [per.tile([P, C], BF16, name=f"hT{m}") for m in range(KH)]
            idxs = [per.tile([P, 1], I32, name=f"idx{j}") for j in range(NB)]
            gcol = [per.tile([P, 1], F32, name=f"gc{j}") for j in range(NB)]
            rwsb = per.tile([P, KD, E], F32, name="rwsb")
            eselsb = per.tile([P, E], F32, name="eselsb")
            ltri = per.tile([P, P], BF16, name="ltri")
            onesm = per.tile([P, P], BF16, name="onesm")
            Sb = per.tile([P, TT], BF16, name="Sb")
            cpt = per.tile([P, TT], F32, name="cpt")
            iotat = per.tile([P, TT], F32, name="iotat")
            ident = per.tile([P, P], BF16, name="ident")
            LG = per.tile([P, TT, E], F32, name="LG")
            S = per.tile([P, TT], F32, name="S")
            gsb = per.tile([P, TT], F32, name="gsb")
            posf = per.tile([P, TT], F32, name="posf")
            pos_i32 = per.tile([P, TT], I32, name="pos_i32")
            ig = per.tile([P, TT, 2], mybir.dt.float16, name="ig")
            iotac = per.tile([P, C], mybir.dt.float16, name="iotac")

            def dma_weights():
                # split by m-half: each half's WAR releases at its last L1/L3
                # read, so the reload streams during the other half + L2
                for half in range(2):
                    sl = slice(half * (H // 2), (half + 1) * (H // 2))
                    for k in range(KD):
                        nc.sync.dma_start(w1sb[k][:, sl], w1_h[ts(k, P), sl])
                        nc.scalar.dma_start(w3sb[k][:, sl], w3_h[ts(k, P), sl])
                for m in range(KH):
                    eng = nc.sync if m % 2 == 0 else nc.scalar
                    eng.dma_start(w2sb[m], w2_h[ts(m, P), :])

            def dma_xT():
                for k in range(KD):
                    nc.gpsimd.dma_start(xTsb[k], xT_h[ts(k, P), :])

            def router_mms():
                """Logit matmuls (PE) + PSUM->SBUF evictions into LG."""
                for i in range(TT):
                    lg = aux.tile([P, E], F32, tag="aux", name=f"lg{i}")
                    for k in range(KD):
                        nc.tensor.matmul(
                            lg,
                            lhsT=xTsb[k][:, ts(i, P)],
                            rhs=rwsb[:, k, :],
                            start=(k == 0),
                            stop=(k == KD - 1),
                        )
                    nc.scalar.activation(LG[:, i, :], lg, ACTF.Copy)

            def compact_mms():
                nc.vector.tensor_copy(Sb, S)
                Aps = aux.tile([P, TT], F32, tag="aux", name="Aps")
                nc.tensor.matmul(Aps, lhsT=ltri, rhs=Sb, start=True, stop=True)
                Bps = aux.tile([P, TT], F32, tag="aux", name="Bps")
                nc.tensor.matmul(Bps, lhsT=onesm, rhs=Sb, start=True, stop=True)
                return Aps, Bps

            def invert_and_gather():
                """Slot-major [token|gate] via PE: tg_ps[s, :] =
                sum_k psel_k[:, block].T @ [iota|gate]_k. Unfilled slots get
                token 0 / gate 0 (psel column all-zero), which is benign."""
                nc.vector.tensor_copy(ig[:, :, 1], gsb)
                tg_ps = pix.tile([P, 16], F32, tag="tgps", name="tg_ps")
                for k in range(TT):
                    pk = psl.tile([P, C], mybir.dt.float16, tag="pk", name="pk")
                    nc.vector.tensor_scalar(
                        pk, iotac, posf[:, k : k + 1], None, op0=ALU.is_equal
                    )
                    for j, js in enumerate(JS):
                        nc.tensor.matmul(
                            tg_ps[:, 2 * j : 2 * j + 2],
                            lhsT=pk[:, js : js + P],
                            rhs=ig[:, k, :],
                            start=(k == 0 and j == 0),
                            stop=(k == TT - 1 and j == NB - 1),
                            skip_group_check=True,
                        )
                for j in range(NB):
                    nc.vector.tensor_copy(idxs[j], tg_ps[:, 2 * j : 2 * j + 1])
                    nc.vector.tensor_copy(gcol[j], tg_ps[:, 2 * j + 1 : 2 * j + 2])
                for j in range(NB):
                    nc.gpsimd.indirect_dma_start(
                        out=xg[j][:, :],
                        out_offset=None,
                        in_=x_h[:, :],
                        in_offset=bass.IndirectOffsetOnAxis(ap=idxs[j][:, :1], axis=0),
                        bounds_check=T - 1,
                        oob_is_err=False,
                    )

            def body():
                # -- weight reloads (self-ordered by WAR on w tiles) --
                dma_weights()

                # -- head: scale gathered tokens by gate, transpose to xgT --
                for j in range(NB):
                    nc.vector.tensor_scalar_mul(xg[j], xg[j], gcol[j])
                for j, js in enumerate(JS):
                    for k in range(KD):
                        pt = aux.tile([P, P], BF16, tag="aux", name="pt")
                        nc.tensor.transpose(pt, xg[j][:, ts(k, P)], ident)
                        nc.scalar.activation(
                            xgT[k][:, js : js + P], pt, ACTF.Copy
                        )

                # -- L1/L3 with router interleaved --
                for m in range(KH):
                    if m == 1:
                        router_mms()
                        dma_xT()  # overwrite xTsb for the next body (WAR)
                    if m == 3:
                        _softmax_top2(nc, rsb, LG, eselsb, S, gsb)
                    for c0, cw in CHK:
                        p1 = p13.tile([P, cw], F32, tag="p13", name="p1")
                        p3 = p13.tile([P, cw], F32, tag="p13", name="p3")
                        for k in range(KD):
                            nc.tensor.matmul(
                                p1,
                                lhsT=w1sb[k][:, ts(m, P)],
                                rhs=xgT[k][:, c0 : c0 + cw],
                                start=(k == 0),
                                stop=(k == KD - 1),
                            )
                        for k in range(KD):
                            nc.tensor.matmul(
                                p3,
                                lhsT=w3sb[k][:, ts(m, P)],
                                rhs=xgT[k][:, c0 : c0 + cw],
                                start=(k == 0),
                                stop=(k == KD - 1),
                            )
                        u13 = sml.tile([P, cw], BF16, tag="u13", name="u13")
                        nc.scalar.activation(u13, p1, ACTF.Silu)
                        nc.vector.tensor_tensor(
                            hT[m][:, c0 : c0 + cw], u13, p3, op=ALU.mult
                        )

                # -- index pipeline for the next body --
                Aps, Bps = compact_mms()
                _compact_scan(nc, rsb, S, cpt, Aps, Bps, posf, pos_i32, pos_h)

                # -- L2: y = hT.T @ w2, hT stationary, two D-halves --
                for j, js in enumerate(JS):
                    if j == 1:
                        invert_and_gather()
                    pys = [
                        pyp.tile([P, NC2], F32, tag="py", name=f"py{n2}")
                        for n2 in range(2)
                    ]
                    for m in range(KH):
                        for n2 in range(2):
                            nc.tensor.matmul(
                                pys[n2],
                                lhsT=hT[m][:, js : js + P],
                                rhs=w2sb[m][:, ts(n2, NC2)],
                                start=(m == 0),
                                stop=(m == KH - 1),
                            )
                    for n2 in range(2):
                        ysb = ysml.tile([P, NC2], BF16, tag="ysb", name="ysb")
                        nc.scalar.activation(ysb, pys[n2], ACTF.Copy)
                        nc.scalar.dma_start(yg_h[js : js + P, ts(n2, NC2)], ysb)

            # ---- prologue: consts, zero tokg, first router chain ----
            nc.scalar.dma_start(rwsb, rw_h.rearrange("(k p) e -> p k e", p=P))
            nc.scalar.dma_start(eselsb, esel_h)
            nc.scalar.dma_start(ltri, ltri_h)
            nc.scalar.dma_start(onesm, onesm_h)
            nc.scalar.dma_start(cpt, cpt_h)
            nc.scalar.dma_start(iotat, iota_h)
            nc.scalar.dma_start(ident, ident_h)
            nc.scalar.dma_start(iotac, iotac_h)
            nc.vector.tensor_copy(ig[:, :, 0], iotat)
            dma_xT()
            router_mms()
            _softmax_top2(nc, rsb, LG, eselsb, S, gsb)
            Aps, Bps = compact_mms()
            _compact_scan(nc, rsb, S, cpt, Aps, Bps, posf, pos_i32, pos_h)
            invert_and_gather()

            hint = (
                mybir.EngineType.PE,
                mybir.EngineType.DVE,
                mybir.EngineType.Activation,
                mybir.EngineType.SP,
                mybir.EngineType.Pool,
            )
            UNROLL = 8 if reps % 8 == 0 else 1
            loop_cm = (
                tc.For_i(0, reps // UNROLL, 1, hint_engines=hint)
                if reps > 1
                else contextlib.nullcontext()
            )
            with loop_cm:
                for _u in range(UNROLL if reps > 1 else 1):
                    body()
    nc.compile()
    return nc


_NC_CACHE = None


def _get_nc():
    global _NC_CACHE
    if _NC_CACHE is None:
        _NC_CACHE = build_moe_nc()
    return _NC_CACHE


def make_in_maps(x, router_w, w1, w2, w3):
    xt = np.ascontiguousarray(np.asarray(x, np.float32).reshape(T, D))
    xT = np.ascontiguousarray(xt.T)
    rw = np.ascontiguousarray(np.asarray(router_w, np.float32))
    w1b = np.asarray(w1).astype(ml_dtypes.bfloat16)
    w2b = np.asarray(w2).astype(ml_dtypes.bfloat16)
    w3b = np.asarray(w3).astype(ml_dtypes.bfloat16)
    ltri = np.triu(np.ones((P, P), ml_dtypes.bfloat16), k=1)
    onesm = np.ones((P, P), ml_dtypes.bfloat16)
    iota = (np.arange(TT)[None, :] * P + np.arange(P)[:, None]).astype(np.float32)
    cpt = (TRASH + iota).astype(np.float32)
    ident = np.eye(P, dtype=ml_dtypes.bfloat16)
    iotac = np.broadcast_to(
        np.arange(C, dtype=np.float16), (P, C)
    ).copy()
    in_maps = []
    for e in range(E):
        esel = np.zeros((P, E), np.float32)
        esel[:, e] = 1.0
        in_maps.append(
            {
                "xT": xT,
                "x": xt,
                "rw": rw,
                "esel": esel,
                "ltri": ltri,
                "onesm": onesm,
                "cpt": cpt,
                "iota": iota,
                "ident": ident,
                "iotac": iotac,
                "w1": np.ascontiguousarray(w1b[e]),
                "w3": np.ascontiguousarray(w3b[e]),
                "w2": np.ascontiguousarray(w2b[e]),
            }
        )
    return in_maps


def combine(results, inputs=None):
    """Sum per-core compact outputs back to token positions.

    pos semantics: < C -> kept at that slot; in [C, TRASH) -> selected but
    dropped (capacity overflow; recompute on host); >= TRASH -> not selected.
    """
    out = np.zeros((T, D), np.float32)
    t_idx = (np.arange(TT)[None, :] * P + np.arange(P)[:, None]).astype(np.int64)
    for e in range(E):
        posv = results[e]["pos"].astype(np.int64)
        selm = posv < C
        y = np.asarray(results[e]["y_g"], np.float32)
        out[t_idx[selm]] += y[posv[selm]]
        dropm = (posv >= C) & (posv < TRASH)
        if dropm.any() and inputs is not None:
            xt = np.asarray(inputs["x"], np.float32).reshape(T, D)
            rw = np.asarray(inputs["router_w"], np.float32)
            tt = t_idx[dropm]
            lg = xt[tt] @ rw
            p = np.exp(lg - lg.max(-1, keepdims=True))
            p /= p.sum(-1, keepdims=True)
            xs = xt[tt] * p[:, e : e + 1]
            a1 = xs @ np.asarray(inputs["w1"][e], np.float32)
            a3 = xs @ np.asarray(inputs["w3"][e], np.float32)
            h = (a1 / (1 + np.exp(-a1))) * a3
            out[tt] += h @ np.asarray(inputs["w2"][e], np.float32)
    return out


def kernel(x, router_w, w1, w2, w3, top_k):
    assert int(top_k) == 2
    nc = _get_nc()
    in_maps = make_in_maps(x, router_w, w1, w2, w3)
    res = run_bass_kernel_spmd(nc, in_maps, list(range(E))).results
    inputs = dict(x=x, router_w=router_w, w1=w1, w2=w2, w3=w3)
    return combine(res, inputs).reshape(2, T // 2, D)

